# revision 10
# baseline (speedup 1.0000x reference)
"""Trainium2 Bass kernel for nn_CriticNetwork (GCN critic head), 8 cores.

Math (reference): h = GCNConv(x, edge_index); sv = relu(h[agent_idx]);
sv = relu(LN(sv@W1+b1)); sv = LN(sv@W2+b2); q = relu(sv + action@Wa+ba) @ Wq + bq.

Exact algebraic restructurings (no approximation):
  * GCNConv is linear-then-propagate, so aggregate in the 128-d INPUT space
    and apply Wg after:  z[v] = sum_{e:dst=v} norm_e * x[src_e].  Only agent
    rows are used downstream, so only edges landing on agent nodes are
    aggregated (~139k slots of 808k).
  * Per-edge norm scaling + segment-sum fuse into one PE matmul per 128-slot
    tile:  zT += G_t^T @ S_t with G_t = host-gathered x rows [slot, feat] and
    S_t[slot, agent] = norm.  Output is directly transposed ([feat, agent]),
    which the whole MLP consumes.
  * LN pre-biases are applied as zero-mean offsets c = b - mean(b); LN means
    come from matmuls against column-mean weights (w1bar from hT, w2bar from
    sv1) concurrent with the main GEMMs; (y + c - mu) evacuates from PSUM in
    ONE scalar_tensor_tensor op; relu(e*r) = (e max 0) * r fuses likewise;
    ba folds into the av evacuation bias.

Perf structure: all matmul operands bf16 (fp32 PSUM accumulate); 32-agent
aggregation chunks with per-chunk variable tile counts (no padding waste);
ALL chunk DMAs issued eagerly up front on the sync+gpsimd queues and weights
on the scalar queue so the HBM stream runs at full rate; a short N=512
warm-up matmul spin holds the PE HAM un-throttled through the initial DMA
wait; the MLP runs as 4 pipelined 256-agent blocks whose PE work is
hand-interleaved with aggregation chunks so the in-order PE queue never
starves; elementwise work is split across ACT/DVE/GPSIMD.

Sharding: agents split 1024/core (data parallel); weights replicated.
"""

import numpy as np
import ml_dtypes

import concourse.bass as bass
import concourse.mybir as mybir
import concourse.tile as tile
from concourse.bass_utils import run_bass_kernel_spmd

BF16 = ml_dtypes.bfloat16

N_NODES = 50000
D_IN = 128
D_HID = 256
FC1 = 512
FC2 = 256
N_ACT = 64
N_AGENTS = 8192
LN_EPS = 1e-5

N_CORES = 8
A_PER_CORE = N_AGENTS // N_CORES        # 1024
ABLK = 256                              # agent block width for MLP
N_ABLK = A_PER_CORE // ABLK             # 4
AGG_CHUNK = 32                          # agents per aggregation chunk
N_CHUNKS = A_PER_CORE // AGG_CHUNK      # 32
CPB = N_CHUNKS // N_ABLK                # chunks per block = 8
N_WARM = 8                              # HAM warm-up matmuls (N=512)

# packed 128-partition weight blob column offsets
WB_WG = 0
WB_W1 = WB_WG + D_HID                   # 256
WB_W2 = WB_W1 + 2 * FC1                 # 1280
WB_WQ = WB_W2 + 4 * FC2                 # 2304
WB_W1BAR = WB_WQ + 2                    # 2306
WB_W2BAR = WB_W1BAR + 2 * 128           # 2562
WB_COLS = WB_W2BAR + 4 * 128            # 3074

# biasT column offsets (fp32, per-partition)
BT_BG = 0        # 2 cols
BT_C1 = 2        # 4 cols
BT_C2 = 6        # 2 cols
BT_BA = 8        # 2 cols
BT_BQ = 10       # 1 col
BT_G1 = 11       # 4
BT_BE1 = 15      # 4
BT_G2 = 19      # 2
BT_BE2 = 21      # 2
BT_COLS = 23

FLOAT = mybir.dt.float32
BF = mybir.dt.bfloat16
AF = mybir.ActivationFunctionType
OP = mybir.AluOpType


def _split_multi_waits(nc, max_waits=1):
    """This container's walrus rejects >1 sync-wait per instruction; move
    extras onto same-engine NoOps inserted right before (equivalent)."""
    for func in nc.m.functions:
        for bb in func.blocks:
            out, changed = [], False
            for inst in bb.instructions:
                si = inst.sync_info
                if si is not None and len(si.on_wait) > max_waits:
                    waits = list(si.on_wait)
                    extra, keep = waits[:-max_waits], waits[-max_waits:]
                    for k in range(0, len(extra), max_waits):
                        nop = mybir.InstNoOp(
                            name=nc.get_next_instruction_name(),
                            engine=inst.engine, bass_nofuse=True,
                            sync_info=mybir.SyncInfo(
                                on_wait=list(extra[k:k + max_waits]),
                                on_update=[]))
                        nc.register_instruction(nop)
                        out.append(nop)
                        changed = True
                    si.on_wait.clear()
                    si.on_wait.extend(keep)
                    inst.sync_info = si
                out.append(inst)
            if changed:
                bb.instructions = out


def _rep3(ap, n):
    """[128, W] AP -> [128, n, W] free-dim repeat (stride 0)."""
    return bass.AP(ap.tensor, ap.offset, [ap.ap[0], [0, n], ap.ap[-1]])


def _as3(ap, n):
    """[128, n*W] AP -> [128, n, W] reshape."""
    return ap.rearrange('p (o w) -> p o w', o=n)


def _build_program(tiles_per_chunk, affine_trivial):
    """tiles_per_chunk: list of N_CHUNKS ints (ceil(slots/128) per chunk)."""
    nc = bass.Bass(target_bir_lowering=False)

    gs_off = []                      # per-chunk col offset into gs blob
    off = 0
    for t in tiles_per_chunk:
        gs_off.append(off)
        off += t * (128 + AGG_CHUNK)
    GS_COLS = off

    gs_t = nc.declare_dram_parameter('gs', [128, GS_COLS], BF, isOutput=False)
    wb128_t = nc.declare_dram_parameter('wb128', [128, WB_COLS], BF,
                                        isOutput=False)
    wb64_t = nc.declare_dram_parameter('wb64', [N_ACT, FC2 + A_PER_CORE], BF,
                                       isOutput=False)
    biasT_t = nc.declare_dram_parameter('biasT', [128, BT_COLS], FLOAT,
                                        isOutput=False)
    q_out = nc.declare_dram_parameter('q', [1, A_PER_CORE], FLOAT,
                                      isOutput=True)

    with tile.TileContext(nc) as tc:
        with (
            tc.tile_pool(name='const', bufs=1) as constp,
            tc.tile_pool(name='ztp', bufs=1) as ztp,
            tc.tile_pool(name='ps_y1', bufs=1, space='PSUM') as ps_y1,
            tc.tile_pool(name='ps_y2', bufs=2, space='PSUM') as ps_y2,
            tc.tile_pool(name='ps_sm', bufs=2, space='PSUM') as ps_sm,
            tc.tile_pool(name='ps_z', bufs=2, space='PSUM') as ps_z,
            tc.tile_pool(name='mlp', bufs=2) as mlp,
            tc.tile_pool(name='keep', bufs=1) as keep,
        ):
            # ---------------- small device-built constants ----------------
            ones1 = constp.tile([128, 128], BF)
            nc.vector.memset(ones1[:], 1.0 / FC1)
            ones2 = constp.tile([128, 128], BF)
            nc.vector.memset(ones2[:], 1.0 / FC2)
            eps_col = constp.tile([128, 1], FLOAT)
            nc.vector.memset(eps_col[:], LN_EPS)
            warm_rhs = constp.tile([128, 512], BF)
            nc.vector.memset(warm_rhs[:], 0.0)

            # HAM warm-up: high-duty N=512 matmuls hold the PE un-throttled
            # while the first gather chunks stream in.  Result never read.
            warm = ps_y2.tile([128, 512], FLOAT, tag='y2', name='warm')
            for i in range(N_WARM):
                nc.tensor.matmul(out=warm[:], lhsT=ones1[:],
                                 rhs=warm_rhs[:], start=(i == 0),
                                 stop=(i == N_WARM - 1))

            # ---------------- input tiles ----------------
            gs = constp.tile([128, GS_COLS], BF)
            wb = constp.tile([128, WB_COLS], BF)
            wg = wb[:, WB_WG:WB_WG + D_HID]
            w1 = wb[:, WB_W1:WB_W1 + 2 * FC1]
            w2 = wb[:, WB_W2:WB_W2 + 4 * FC2]
            wq = wb[:, WB_WQ:WB_WQ + 2]
            w1bar = wb[:, WB_W1BAR:WB_W1BAR + 256]
            w2bar = wb[:, WB_W2BAR:WB_W2BAR + 512]
            wb64 = constp.tile([N_ACT, FC2 + A_PER_CORE], BF)
            wa = wb64[:, 0:FC2]
            actT = wb64[:, FC2:FC2 + A_PER_CORE]
            biasT = constp.tile([128, BT_COLS], FLOAT)
            bgT = biasT[:, BT_BG:BT_BG + 2]
            c1T = biasT[:, BT_C1:BT_C1 + 4]
            c2T = biasT[:, BT_C2:BT_C2 + 2]
            baT = biasT[:, BT_BA:BT_BA + 2]
            bq_sb = biasT[0:1, BT_BQ:BT_BQ + 1]
            g1T = biasT[:, BT_G1:BT_G1 + 4]
            be1T = biasT[:, BT_BE1:BT_BE1 + 4]
            g2T = biasT[:, BT_G2:BT_G2 + 2]
            be2T = biasT[:, BT_BE2:BT_BE2 + 2]

            # eager DMA: all chunks on sync/gpsimd queues, weights on scalar
            for c in range(N_CHUNKS):
                w = tiles_per_chunk[c] * (128 + AGG_CHUNK)
                eng = nc.sync if (c % 2 == 0) else nc.gpsimd
                eng.dma_start(out=gs[:, gs_off[c]:gs_off[c] + w],
                              in_=gs_t[:, gs_off[c]:gs_off[c] + w])
            nc.scalar.dma_start(out=wb[:], in_=wb128_t[:])
            nc.scalar.dma_start(out=wb64[:], in_=wb64_t[:])
            nc.scalar.dma_start(out=biasT[:], in_=biasT_t[:])

            # ------------- aggregation -------------
            zt = [ztp.tile([D_IN, ABLK], BF, tag=f'zt{b}', name=f'zt{b}')
                  for b in range(N_ABLK)]
            z_cur = {}

            def emit_chunk(c):
                """Aggregate chunk c into its block's z PSUM tile."""
                b, ci = divmod(c, CPB)
                if ci == 0:
                    z_cur['t'] = ps_z.tile([D_IN, ABLK], FLOAT, tag='z',
                                           name='z')
                z_ps = z_cur['t']
                t_c = tiles_per_chunk[c]
                o = gs_off[c]
                zsl = z_ps[:, ci * AGG_CHUNK:(ci + 1) * AGG_CHUNK]
                for k in range(t_c):
                    nc.tensor.matmul(
                        out=zsl,
                        lhsT=gs[:, o + k * 128:o + (k + 1) * 128],
                        rhs=gs[:, o + t_c * 128 + k * AGG_CHUNK:
                               o + t_c * 128 + (k + 1) * AGG_CHUNK],
                        start=(k == 0),
                        stop=(k == t_c - 1))
                if ci == CPB - 1:
                    nc.vector.tensor_copy(out=zt[b][:], in_=z_ps[:])

            # ------------- MLP block (transposed activations) -------------
            def mlp_block(b):
                """Generator yielding between PE-heavy stages.  Stage map:
                A: av + h matmuls, hT evac, mu1 matmuls
                B: y1 matmuls, mu1 evac, e1, sq1
                D: var1 matmuls, r1, sv1
                F: y2 + mu2 matmuls, mu2 evac, e2, sq2
                G: var2 matmuls, r2, t2, sav, savr
                I: q matmuls + output DMA
                """
                asl = slice(b * ABLK, (b + 1) * ABLK)
                # --- A ---
                av_ps = ps_y2.tile([128, 2 * ABLK], FLOAT, tag='y2',
                                   name='av_ps')
                for o in range(2):
                    nc.tensor.matmul(out=av_ps[:, o * ABLK:(o + 1) * ABLK],
                                     lhsT=wa[:, o * 128:(o + 1) * 128],
                                     rhs=actT[:, asl], start=True, stop=True)
                h_ps = ps_y2.tile([128, 2 * ABLK], FLOAT, tag='y2',
                                  name='h_ps')
                for o in range(2):
                    nc.tensor.matmul(out=h_ps[:, o * ABLK:(o + 1) * ABLK],
                                     lhsT=wg[:, o * 128:(o + 1) * 128],
                                     rhs=zt[b][:], start=True, stop=True)
                avT = mlp.tile([128, 2 * ABLK], BF, tag='avT', name='avT')
                nc.scalar.activation(out=avT[:, 0:ABLK], in_=av_ps[:, 0:ABLK],
                                     func=AF.Identity, bias=baT[:, 0:1],
                                     scale=1.0)
                nc.vector.tensor_scalar_add(out=avT[:, ABLK:2 * ABLK],
                                            in0=av_ps[:, ABLK:2 * ABLK],
                                            scalar1=baT[:, 1:2])
                hT = mlp.tile([128, 2 * ABLK], BF, tag='hT', name='hT')
                nc.scalar.activation(out=hT[:, 0:ABLK], in_=h_ps[:, 0:ABLK],
                                     func=AF.Relu, bias=bgT[:, 0:1],
                                     scale=1.0)
                nc.vector.tensor_scalar(
                    out=hT[:, ABLK:2 * ABLK], in0=h_ps[:, ABLK:2 * ABLK],
                    scalar1=bgT[:, 1:2], scalar2=0.0, op0=OP.add, op1=OP.max)
                mu1_ps = ps_sm.tile([128, ABLK], FLOAT, tag='sm',
                                    name='mu1_ps')
                for k in range(2):
                    nc.tensor.matmul(out=mu1_ps[:],
                                     lhsT=w1bar[:, k * 128:(k + 1) * 128],
                                     rhs=hT[:, k * ABLK:(k + 1) * ABLK],
                                     start=(k == 0), stop=(k == 1))
                yield
                # --- B ---
                y1_ps = ps_y1.tile([128, 4 * ABLK], FLOAT, tag='y1',
                                   name='y1_ps')
                for o in range(4):
                    for k in range(2):
                        nc.tensor.matmul(
                            out=y1_ps[:, o * ABLK:(o + 1) * ABLK],
                            lhsT=w1[:, (k * 4 + o) * 128:(k * 4 + o + 1) * 128],
                            rhs=hT[:, k * ABLK:(k + 1) * ABLK],
                            start=(k == 0), stop=(k == 1))
                mu1_sb = mlp.tile([128, ABLK], BF, tag='mu1', name='mu1_sb')
                nc.vector.tensor_copy(out=mu1_sb[:], in_=mu1_ps[:])
                # e1_o = (y1_o + c1_o) - mu1, straight out of PSUM
                e1 = mlp.tile([128, 4 * ABLK], BF, tag='e1', name='e1')
                for o in range(4):
                    nc.vector.scalar_tensor_tensor(
                        out=e1[:, o * ABLK:(o + 1) * ABLK],
                        in0=y1_ps[:, o * ABLK:(o + 1) * ABLK],
                        scalar=c1T[:, o:o + 1], in1=mu1_sb[:],
                        op0=OP.add, op1=OP.subtract)
                sq1 = mlp.tile([128, 4 * ABLK], BF, tag='sq1', name='sq1')
                nc.scalar.activation(out=sq1[:, 0:2 * ABLK],
                                     in_=e1[:, 0:2 * ABLK], func=AF.Square)
                nc.gpsimd.tensor_tensor(out=sq1[:, 2 * ABLK:],
                                        in0=e1[:, 2 * ABLK:],
                                        in1=e1[:, 2 * ABLK:], op=OP.mult)
                yield
                # --- D ---
                var1 = ps_sm.tile([128, ABLK], FLOAT, tag='sm', name='var1')
                for o in range(4):
                    nc.tensor.matmul(out=var1[:], lhsT=ones1[:],
                                     rhs=sq1[:, o * ABLK:(o + 1) * ABLK],
                                     start=(o == 0), stop=(o == 3))
                lg1 = mlp.tile([128, ABLK], FLOAT, tag='lg1', name='lg1')
                nc.scalar.activation(out=lg1[:], in_=var1[:], func=AF.Ln,
                                     bias=eps_col[:, 0:1])
                r1 = mlp.tile([128, ABLK], BF, tag='r1', name='r1')
                nc.scalar.activation(out=r1[:], in_=lg1[:], func=AF.Exp,
                                     scale=-0.5)
                sv1 = mlp.tile([128, 4 * ABLK], BF, tag='sv1', name='sv1')
                if affine_trivial:
                    # relu(e*r) = (e max 0) * r  (r > 0)
                    nc.vector.scalar_tensor_tensor(
                        out=_as3(sv1[:], 4), in0=_as3(e1[:], 4),
                        scalar=0.0, in1=_rep3(r1[:], 4),
                        op0=OP.max, op1=OP.mult)
                else:
                    t1 = mlp.tile([128, 4 * ABLK], BF, tag='t1', name='t1')
                    nc.vector.tensor_tensor(
                        out=_as3(t1[:], 4), in0=_as3(e1[:], 4),
                        in1=_rep3(r1[:], 4), op=OP.mult)
                    for o in range(4):
                        nc.scalar.activation(
                            out=sv1[:, o * ABLK:(o + 1) * ABLK],
                            in_=t1[:, o * ABLK:(o + 1) * ABLK],
                            func=AF.Relu, bias=be1T[:, o:o + 1],
                            scale=g1T[:, o:o + 1])
                yield
                # --- F ---
                y2_ps = ps_y2.tile([128, 2 * ABLK], FLOAT, tag='y2',
                                   name='y2_ps')
                for o in range(2):
                    for k in range(4):
                        nc.tensor.matmul(
                            out=y2_ps[:, o * ABLK:(o + 1) * ABLK],
                            lhsT=w2[:, (k * 2 + o) * 128:(k * 2 + o + 1) * 128],
                            rhs=sv1[:, k * ABLK:(k + 1) * ABLK],
                            start=(k == 0), stop=(k == 3))
                mu2_ps = ps_sm.tile([128, ABLK], FLOAT, tag='sm',
                                    name='mu2_ps')
                for k in range(4):
                    nc.tensor.matmul(out=mu2_ps[:],
                                     lhsT=w2bar[:, k * 128:(k + 1) * 128],
                                     rhs=sv1[:, k * ABLK:(k + 1) * ABLK],
                                     start=(k == 0), stop=(k == 3))
                mu2_sb = mlp.tile([128, ABLK], BF, tag='mu2', name='mu2_sb')
                nc.vector.tensor_copy(out=mu2_sb[:], in_=mu2_ps[:])
                e2 = mlp.tile([128, 2 * ABLK], BF, tag='e2', name='e2')
                for o in range(2):
                    nc.vector.scalar_tensor_tensor(
                        out=e2[:, o * ABLK:(o + 1) * ABLK],
                        in0=y2_ps[:, o * ABLK:(o + 1) * ABLK],
                        scalar=c2T[:, o:o + 1], in1=mu2_sb[:],
                        op0=OP.add, op1=OP.subtract)
                sq2 = mlp.tile([128, 2 * ABLK], BF, tag='sq2', name='sq2')
                nc.gpsimd.tensor_tensor(out=sq2[:], in0=e2[:], in1=e2[:],
                                        op=OP.mult)
                yield
                # --- G ---
                var2 = ps_sm.tile([128, ABLK], FLOAT, tag='sm', name='var2')
                for o in range(2):
                    nc.tensor.matmul(out=var2[:], lhsT=ones2[:],
                                     rhs=sq2[:, o * ABLK:(o + 1) * ABLK],
                                     start=(o == 0), stop=(o == 1))
                lg2 = mlp.tile([128, ABLK], FLOAT, tag='lg2', name='lg2')
                nc.scalar.activation(out=lg2[:], in_=var2[:], func=AF.Ln,
                                     bias=eps_col[:, 0:1])
                r2 = mlp.tile([128, ABLK], BF, tag='r2', name='r2')
                nc.scalar.activation(out=r2[:], in_=lg2[:], func=AF.Exp,
                                     scale=-0.5)
                sav = mlp.tile([128, 2 * ABLK], BF, tag='sav', name='sav')
                if affine_trivial:
                    t2 = mlp.tile([128, 2 * ABLK], BF, tag='t2', name='t2')
                    nc.vector.tensor_tensor(
                        out=_as3(t2[:], 2), in0=_as3(e2[:], 2),
                        in1=_rep3(r2[:], 2), op=OP.mult)
                    nc.gpsimd.tensor_tensor(out=sav[:], in0=t2[:],
                                            in1=avT[:], op=OP.add)
                else:
                    t2 = mlp.tile([128, 2 * ABLK], BF, tag='t2', name='t2')
                    nc.vector.tensor_tensor(
                        out=_as3(t2[:], 2), in0=_as3(e2[:], 2),
                        in1=_rep3(r2[:], 2), op=OP.mult)
                    sv2 = mlp.tile([128, 2 * ABLK], BF, tag='sv2',
                                   name='sv2')
                    for o in range(2):
                        nc.scalar.activation(
                            out=sv2[:, o * ABLK:(o + 1) * ABLK],
                            in_=t2[:, o * ABLK:(o + 1) * ABLK],
                            func=AF.Identity, bias=be2T[:, o:o + 1],
                            scale=g2T[:, o:o + 1])
                    nc.gpsimd.tensor_tensor(out=sav[:], in0=sv2[:],
                                            in1=avT[:], op=OP.add)
                savr = mlp.tile([128, 2 * ABLK], BF, tag='savr', name='savr')
                nc.vector.tensor_scalar_max(out=savr[:], in0=sav[:],
                                            scalar1=0.0)
                yield
                # --- I ---
                q_full = ps_sm.tile([128, ABLK], FLOAT, tag='sm', name='q')
                q_ps = q_full[0:1, :]
                for o in range(2):
                    nc.tensor.matmul(out=q_ps,
                                     lhsT=wq[:, o:o + 1],
                                     rhs=savr[:, o * ABLK:(o + 1) * ABLK],
                                     start=(o == 0), stop=(o == 1))
                q_sb = keep.tile([1, ABLK], FLOAT, tag=f'qsb{b}', name='q_sb')
                nc.scalar.activation(out=q_sb[:], in_=q_ps,
                                     func=AF.Identity, bias=bq_sb[:, 0:1])
                nc.sync.dma_start(out=q_out[0:1, b * ABLK:(b + 1) * ABLK],
                                  in_=q_sb[:])
                yield

            # ------------- interleaved emission schedule -------------
            gens = [mlp_block(b) for b in range(N_ABLK)]

            def step(b):
                try:
                    next(gens[b])
                except StopIteration:
                    pass

            C = emit_chunk
            for c in range(4):
                C(c)
            C(4); C(5); C(6); C(7)
            step(0)                      # b0.A  (needs zt0 = chunks 0-7)
            C(8); C(9)
            step(0)                      # b0.B
            C(10); C(11)
            step(0)                      # b0.D
            C(12); C(13)
            step(0)                      # b0.F
            C(14); C(15)
            step(1)                      # b1.A
            step(0)                      # b0.G
            C(16); C(17)
            step(1)                      # b1.B
            step(0)                      # b0.I
            C(18); C(19)
            step(1)                      # b1.D
            C(20); C(21)
            step(1)                      # b1.F
            C(22); C(23)
            step(2)                      # b2.A
            step(1)                      # b1.G
            C(24); C(25)
            step(2)                      # b2.B
            step(1)                      # b1.I
            C(26); C(27)
            step(2)                      # b2.D
            C(28); C(29)
            step(2)                      # b2.F
            C(30); C(31)
            step(3)                      # b3.A
            step(2)                      # b2.G
            step(3)                      # b3.B
            step(2)                      # b2.I
            step(3)                      # b3.D
            step(3)                      # b3.F
            step(3)                      # b3.G
            step(3)                      # b3.I

    _split_multi_waits(nc)
    return nc


_NC_CACHE = {}


def _get_program(tiles_per_chunk, affine_trivial):
    key = (tuple(tiles_per_chunk), affine_trivial)
    if key not in _NC_CACHE:
        _NC_CACHE[key] = _build_program(list(tiles_per_chunk),
                                        affine_trivial)
    return _NC_CACHE[key]


def _pack_agents(s_all):
    """Permute agents so degree (slot) sums balance across cores and pack
    each core's agents into N_CHUNKS 32-agent bins sharing one tile-count
    profile (caps).  Returns (perm [N_AGENTS], caps [N_CHUNKS])."""
    order = np.argsort(-s_all, kind='stable')
    core_lists = [[] for _ in range(N_CORES)]
    for i, a in enumerate(order):               # snake deal across cores
        r, j = divmod(i, N_CORES)
        c = j if r % 2 == 0 else N_CORES - 1 - j
        core_lists[c].append(int(a))
    maxslots = max(int(s_all[l].sum()) for l in core_lists)
    total_tiles = -(-maxslots // 128) + 4
    while True:
        base, extra = divmod(total_tiles, N_CHUNKS)
        caps = [base] * N_CHUNKS
        for i in range(extra):                  # spread big chunks evenly
            caps[(i * N_CHUNKS) // extra] += 1
        packs = []
        ok = True
        for cl in core_lists:                   # cl is desc by slot count
            bins = [[] for _ in range(N_CHUNKS)]
            rem_s = [c * 128 for c in caps]
            rem_a = [AGG_CHUNK] * N_CHUNKS
            for a in cl:
                s = int(s_all[a])
                jb, rb = -1, -1
                for j in range(N_CHUNKS):       # worst-fit w/ reserve guard
                    if (rem_a[j] > 0 and rem_s[j] - s >= rem_a[j] - 1
                            and rem_s[j] > rb):
                        jb, rb = j, rem_s[j]
                if jb < 0:
                    ok = False
                    break
                bins[jb].append(a)
                rem_s[jb] -= s
                rem_a[jb] -= 1
            if not ok:
                break
            packs.append([a for b in bins for a in b])
        if ok:
            perm = np.array([a for p in packs for a in p], dtype=np.int64)
            return perm, caps
        total_tiles += 2


def _host_prep(x, edge_index, action, agent_idx, Wg, bg, W1, b1, g1, be1,
               W2, b2, g2, be2, Wa, ba, Wq, bq):
    """Graph preprocessing + per-core input maps (host: indexing/layout only)."""
    src = np.asarray(edge_index[0], dtype=np.int64)
    dst = np.asarray(edge_index[1], dtype=np.int64)
    agent_idx = np.asarray(agent_idx, dtype=np.int64)

    cnt = np.bincount(dst, minlength=N_NODES)          # in-degree (no self)
    order = np.argsort(dst, kind='stable')
    src_s = src[order]
    indptr = np.zeros(N_NODES + 1, dtype=np.int64)
    np.cumsum(cnt, out=indptr[1:])
    deg = (cnt + 1).astype(np.float64)
    dinv = (1.0 / np.sqrt(deg)).astype(np.float32)

    s_all = (cnt[agent_idx] + 1).astype(np.int64)      # slots per agent
    perm, caps = _pack_agents(s_all)
    agent_idx = agent_idx[perm]

    g1 = np.asarray(g1, np.float32)
    be1 = np.asarray(be1, np.float32)
    g2 = np.asarray(g2, np.float32)
    be2 = np.asarray(be2, np.float32)
    affine_trivial = bool(
        np.all(g1 == 1) and np.all(be1 == 0)
        and np.all(g2 == 1) and np.all(be2 == 0))

    # weights / biases shared by all cores
    W1f = np.asarray(W1, np.float32)
    W2f = np.asarray(W2, np.float32)
    W1s = np.ascontiguousarray(
        W1f.reshape(2, 128, FC1).transpose(1, 0, 2).reshape(128, 2 * FC1))
    W2s = np.ascontiguousarray(
        W2f.reshape(4, 128, FC2).transpose(1, 0, 2).reshape(128, 4 * FC2))
    Wqs = np.ascontiguousarray(np.asarray(Wq, np.float32).reshape(2, 128).T)
    w1bar = W1f.mean(axis=1)  # [256]
    w1bar_rep = np.repeat(w1bar.reshape(2, 128, 1), 128, axis=2) \
        .transpose(1, 0, 2).reshape(128, 256)
    w2bar = W2f.mean(axis=1)  # [512]
    w2bar_rep = np.repeat(w2bar.reshape(4, 128, 1), 128, axis=2) \
        .transpose(1, 0, 2).reshape(128, 512)
    wb128 = np.zeros((128, WB_COLS), dtype=np.float32)
    wb128[:, WB_WG:WB_WG + D_HID] = Wg
    wb128[:, WB_W1:WB_W1 + 2 * FC1] = W1s
    wb128[:, WB_W2:WB_W2 + 4 * FC2] = W2s
    wb128[:, WB_WQ:WB_WQ + 2] = Wqs
    wb128[:, WB_W1BAR:WB_W1BAR + 256] = w1bar_rep
    wb128[:, WB_W2BAR:WB_W2BAR + 512] = w2bar_rep
    wb128 = wb128.astype(BF16)

    action = np.asarray(action, dtype=np.float32)

    b1 = np.asarray(b1, np.float32)
    b2 = np.asarray(b2, np.float32)
    c1 = b1 - b1.mean()
    c2 = b2 - b2.mean()

    biasT = np.zeros((128, BT_COLS), dtype=np.float32)
    biasT[:, BT_BG:BT_BG + 2] = np.asarray(bg, np.float32).reshape(2, 128).T
    biasT[:, BT_C1:BT_C1 + 4] = c1.reshape(4, 128).T
    biasT[:, BT_C2:BT_C2 + 2] = c2.reshape(2, 128).T
    biasT[:, BT_BA:BT_BA + 2] = np.asarray(ba, np.float32).reshape(2, 128).T
    biasT[0, BT_BQ] = np.float32(np.asarray(bq).reshape(-1)[0])
    biasT[:, BT_G1:BT_G1 + 4] = g1.reshape(4, 128).T
    biasT[:, BT_BE1:BT_BE1 + 4] = be1.reshape(4, 128).T
    biasT[:, BT_G2:BT_G2 + 2] = g2.reshape(2, 128).T
    biasT[:, BT_BE2:BT_BE2 + 2] = be2.reshape(2, 128).T

    x_b = np.ascontiguousarray(x, dtype=np.float32).astype(BF16)
    action = action[perm]

    in_maps = []
    for core in range(N_CORES):
        a0 = core * A_PER_CORE
        chunks = []
        for c in range(N_CHUNKS):
            t_c = caps[c]
            v = agent_idx[a0 + c * AGG_CHUNK: a0 + (c + 1) * AGG_CHUNK]
            l = cnt[v]
            L = int(l.sum())
            # edge slots: concatenated CSR spans of each agent's node
            ofs = np.repeat(
                indptr[v] - np.concatenate(([0], np.cumsum(l)[:-1])), l)
            epos = np.arange(L, dtype=np.int64) + ofs
            e_src = src_s[epos]
            e_acol = np.repeat(np.arange(AGG_CHUNK), l)
            e_norm = dinv[e_src] * dinv[np.repeat(v, l)]
            # self slots appended
            srcs = np.concatenate([e_src, v])
            acol = np.concatenate([e_acol, np.arange(AGG_CHUNK)])
            norm = np.concatenate([e_norm, dinv[v] * dinv[v]])
            n_slots = L + AGG_CHUNK
            assert n_slots <= t_c * 128, (n_slots, t_c)
            sid = np.zeros(t_c * 128, dtype=np.int64)
            sid[:n_slots] = srcs
            g_part = x_b[sid.reshape(t_c, 128).T].reshape(128, t_c * 128)
            sm = np.zeros((t_c * 128, AGG_CHUNK), dtype=np.float32)
            sm[np.arange(n_slots), acol] = norm
            s_part = sm.reshape(t_c, 128, AGG_CHUNK) \
                .transpose(1, 0, 2).reshape(128, t_c * AGG_CHUNK).astype(BF16)
            chunks.append(np.concatenate([g_part, s_part], axis=1))
        gs = np.ascontiguousarray(np.concatenate(chunks, axis=1))
        wb64 = np.zeros((N_ACT, FC2 + A_PER_CORE), dtype=np.float32)
        wb64[:, 0:FC2] = Wa
        wb64[:, FC2:] = action[a0:a0 + A_PER_CORE].T
        in_maps.append({
            'gs': gs,
            'wb128': wb128, 'wb64': wb64.astype(BF16),
            'biasT': biasT,
        })
    return in_maps, caps, perm, affine_trivial


_LAST_EXEC_NS = None


def kernel(trace=False, **inputs):
    global _LAST_EXEC_NS
    inputs = {k: np.asarray(v) for k, v in inputs.items()}
    in_maps, caps, perm, affine_trivial = _host_prep(**inputs)
    nc = _get_program(caps, affine_trivial)
    res = run_bass_kernel_spmd(nc, in_maps, core_ids=list(range(N_CORES)),
                               trace=trace)
    _LAST_EXEC_NS = res.exec_time_ns
    q = np.concatenate([res.results[i]['q'][0] for i in range(N_CORES)])
    out = np.empty((N_AGENTS,), dtype=np.float32)
    out[perm] = q                      # undo the packing permutation
    return out.reshape(N_AGENTS, 1)
